# revision 1
# baseline (speedup 1.0000x reference)
"""StyleGAN2 fused upsample2x + 3x3 conv + FIR(1,3,3,1) + bias — TRN2 Bass kernel.

Math: zero-insert-by-2 -> corr(w, full pad) -> pad1 -> corr(FIR 4x4) composes
into a single stride-2 transposed conv with a 6x6 kernel W6 = fir (*) w.
By output parity (alpha, beta) in {0,1}^2 this splits into 4 ordinary 3x3
SAME convs over the original 64x64 input:

  out[n, o, 2u+a, 2v+b] = sum_{c,dr,dc} K[a,b][o,c,dr,dc] * x[n,c,u+dr,v+dc]
  K[a,b][...,di,dj] = W6[..., e_a[di], f_b[dj]],  e_0=(1,3,5), e_1=(0,2,4)

Each parity conv is 9 shifted matmuls (K=256 split in 2x128) accumulated in
PSUM; o=256 is split in 2x128 (M); spatial 64x64 is tiled as 8 chunks of
8 rows x 64 cols (N=512).  Data-parallel over batch: 2 images per core,
8 cores.  Matmuls run in float32r (fast fp32 mode, 1 cycle/row).
"""

import sys

sys.path.insert(0, "/opt/trn_rl_repo")

import numpy as np

import concourse.bacc as bacc
import concourse.mybir as mybir
import concourse.tile as tile
from concourse.bass_utils import run_bass_kernel_spmd

N_CORES = 8
IMGS = 16
IMG_PER_CORE = IMGS // N_CORES  # 2
C = 256  # in channels
O = 256  # out channels
H = W = 64
NK = C // 128  # 2 contraction splits
NM = O // 128  # 2 output-partition splits
NUB = 8  # row-blocks of 8 output (parity-plane) rows
ROWS_PER_UB = 8
HP = H + 2  # padded rows/cols

_compiled = None
LAST_RESULTS = None


def _build():
    nc = bacc.Bacc(None, target_bir_lowering=False, debug=False)
    dt = mybir.dt

    xp_d = nc.dram_tensor(
        "xp", (IMG_PER_CORE, NK, 128, HP * HP), dt.float32r, kind="ExternalInput"
    )
    wt_d = nc.dram_tensor(
        "wt", (128, 4 * 9 * NK * NM * 128), dt.float32r, kind="ExternalInput"
    )
    b_d = nc.dram_tensor("bias", (128, NM), dt.float32, kind="ExternalInput")
    out_d = nc.dram_tensor(
        "out", (IMG_PER_CORE, O, 2 * H, 2 * W), dt.float32, kind="ExternalOutput"
    )

    with tile.TileContext(nc) as tc:
        with (
            tc.tile_pool(name="xpool", bufs=1) as xpool,
            tc.tile_pool(name="wpool", bufs=1) as wpool,
            tc.tile_pool(name="opool", bufs=3) as opool,
            tc.tile_pool(name="psum", bufs=8, space="PSUM") as psum_pool,
        ):
            wt_t = wpool.tile([128, 4 * 9 * NK * NM * 128], dt.float32r, tag="wt")
            xp_t = {}

            def load_xp(img, k, split=False):
                t = xpool.tile([128, HP, HP], dt.float32r, tag=f"xp{img}{k}")
                src = xp_d.ap()[img, k].rearrange("p (h w) -> p h w", h=HP)
                if split:
                    nc.sync.dma_start(t[:, :24, :], src[:, :24, :])
                    nc.sync.dma_start(t[:, 24:, :], src[:, 24:, :])
                else:
                    nc.sync.dma_start(t[:], src)
                xp_t[img, k] = t

            def load_wt(m, par, ks=(0, 1)):
                # weight layout: [m, par, k, tap, o]
                KC = 9 * 128
                base = (m * 4 + par) * NK * KC
                for k in ks:
                    blk = base + k * KC
                    nc.sync.dma_start(
                        wt_t[:, blk : blk + KC], wt_d.ap()[:, blk : blk + KC]
                    )

            # Order: minimal working set first (k-outer accumulation means the
            # first 9 matmuls touch only xp[0,0] top rows + wt chunk (0,0,k0)).
            load_xp(0, 0, split=True)
            load_wt(0, 0, ks=(0,))
            b_t = wpool.tile([128, NM], dt.float32, tag="bias")
            nc.sync.dma_start(b_t[:], b_d.ap()[:])
            load_xp(0, 1)
            load_wt(0, 0, ks=(1,))
            for par in range(1, 4):
                load_wt(0, par)
            for par in range(4):
                load_wt(1, par)
            load_xp(1, 0)
            load_xp(1, 1)

            for img in range(IMG_PER_CORE):
                for m in range(NM):
                    for ub in range(NUB):
                        u0 = ub * ROWS_PER_UB
                        out_sb = opool.tile([128, 2 * ROWS_PER_UB, 2 * W], dt.float32)
                        out_v = out_sb[:].rearrange(
                            "p (u a) (v b) -> p u a v b", a=2, b=2
                        )
                        for a in range(2):
                            for b in range(2):
                                par = a * 2 + b
                                acc = psum_pool.tile(
                                    [128, ROWS_PER_UB, W], dt.float32
                                )
                                n_acc = 0
                                for k in range(NK):
                                    for di in range(3):
                                        for dj in range(3):
                                            tap = di * 3 + dj
                                            idx = ((m * 4 + par) * NK + k) * 9 + tap
                                            rhs = xp_t[img, k][
                                                :,
                                                u0 + di : u0 + di + ROWS_PER_UB,
                                                dj : dj + W,
                                            ]
                                            nc.tensor.matmul(
                                                acc[:],
                                                wt_t[:, idx * 128 : (idx + 1) * 128],
                                                rhs,
                                                start=(n_acc == 0),
                                                stop=(n_acc == NK * 9 - 1),
                                            )
                                            n_acc += 1
                                nc.scalar.activation(
                                    out_v[:, :, a, :, b],
                                    acc[:],
                                    mybir.ActivationFunctionType.Identity,
                                    bias=b_t[:, m : m + 1],
                                )
                        nc.sync.dma_start(
                            out_d.ap()[
                                img,
                                m * 128 : (m + 1) * 128,
                                2 * u0 : 2 * u0 + 2 * ROWS_PER_UB,
                                :,
                            ],
                            out_sb[:],
                        )

    nc.compile()
    return nc


def _compose_weights(w):
    """w (256,256,3,3) -> Wt (128, 4*9*2*2*128) f32, layout
    [c_local, (par, tap, k, m, o_local)]."""
    k1 = np.array([1.0, 3.0, 3.0, 1.0], dtype=np.float64)
    fir = np.outer(k1, k1) / 16.0
    w64 = w.astype(np.float64)
    W6 = np.zeros((O, C, 6, 6), dtype=np.float64)
    for s in range(4):
        for t in range(4):
            W6[:, :, s : s + 3, t : t + 3] += fir[s, t] * w64
    es = [(1, 3, 5), (0, 2, 4)]
    # K_all[a, b, di, dj, k, m, c_local, o_local]
    K_all = np.empty((2, 2, 3, 3, NK, NM, 128, 128), dtype=np.float32)
    for a in range(2):
        for b in range(2):
            for di in range(3):
                for dj in range(3):
                    sub = W6[:, :, es[a][di], es[b][dj]]  # (o, c)
                    for k in range(NK):
                        for m in range(NM):
                            K_all[a, b, di, dj, k, m] = (
                                sub[m * 128 : (m + 1) * 128, k * 128 : (k + 1) * 128]
                                .T.astype(np.float32)
                            )
    # -> [c_local, m, a, b, k, di, dj, o_local]
    return np.ascontiguousarray(K_all.transpose(6, 5, 0, 1, 4, 2, 3, 7)).reshape(
        128, -1
    )


def kernel(x, w, b):
    global _compiled, LAST_RESULTS
    if _compiled is None:
        _compiled = _build()
    nc = _compiled

    x = np.asarray(x, dtype=np.float32)
    w = np.asarray(w, dtype=np.float32)
    b = np.asarray(b, dtype=np.float32)

    wt = _compose_weights(w)
    b2 = np.ascontiguousarray(b.reshape(NM, 128).T)  # [o_local, m]
    xp = np.pad(x, ((0, 0), (0, 0), (1, 1), (1, 1)))  # (16, 256, 66, 66)
    xp = np.ascontiguousarray(
        xp.reshape(N_CORES, IMG_PER_CORE, NK, 128, HP * HP)
    )

    in_maps = [
        {"xp": xp[core], "wt": wt, "bias": b2} for core in range(N_CORES)
    ]
    try:
        res = run_bass_kernel_spmd(nc, in_maps, list(range(N_CORES)))
    except ModuleNotFoundError:
        # BASS_TRACE set in an env without the axon NTFF hook module —
        # retry with tracing disabled.
        import os

        os.environ["BASS_NEVER_TRACE"] = "1"
        res = run_bass_kernel_spmd(nc, in_maps, list(range(N_CORES)))
    LAST_RESULTS = res
    out = np.concatenate([res.results[i]["out"] for i in range(N_CORES)], axis=0)
    return out

